# revision 16
# baseline (speedup 1.0000x reference)
"""Bidirectional Mamba (MixerModel) Trainium2 kernel — minimal-instruction design.

Sharding: data-parallel over batch. 8 batch elements -> 8 NeuronCores; each
core runs the full 2-direction x 4-layer model for its batch element (the
backward direction consumes a host-flipped input copy; the softmax pool is
order-invariant so its output needs no unflip). Host stacks per-core [64]
outputs.

The NEFF execution cost here is dominated by a fixed per-instruction
overhead, so the kernel is built to minimize instruction count:
 - both directions ride one [128, T] residual tile (dir d on partitions
   64d:64d+64); LN stats for both dirs come from one matmul set against a
   2-column selector, and the row->tile broadcasts (mean/rstd, B/C rows,
   pool weights) go through one DMA each (DRAM bounce + stride-0 source)
   instead of per-row matmul chains;
 - the 16-state selective scan runs as ONE tensor_tensor_scan over a
   [128, 16*(TCH+1)] tile: states are concatenated along the free axis with
   a gap column whose decay is 0, which resets the recurrence to the
   injected per-state carry (dbx gap col = carry);
 - dA/dBx/y are built with 3D stride-0-broadcast APs (dt and u broadcast
   over the state axis, A over time) so each is a single DVE op; the
   sum over states is a 4-level pairwise tree on contiguous halves;
 - dt_w @ xproj_w[:dt_rank] is composed on the host so dt comes straight
   from one matmul on xact.
"""

import numpy as np

D_MODEL = 64
N_LAYER = 4
D_INNER = 128
D_STATE = 16
D_CONV = 4
DT_RANK = 4
EPS = 1e-5
T = 2048
B = 8
NCORES = 8
TCH = 1024             # scan chunk
NCH = T // TCH
L = TCH + 1            # segment length incl. gap column
MM = 512               # max matmul free dim (one PSUM bank)


def _legalize_sync_waits(nc, mybir, maxw=1):
    """This container's walrus only accepts one sync-wait command per
    instruction (newer bass emits several, e.g. on the kernel-tail drain).
    Split excess waits onto preceding same-engine NOPs — semantically
    identical: the engine blocks on each wait in turn before the original
    instruction issues."""
    for blk in nc.m.functions[0].blocks:
        newlist, changed = [], False
        for inst in blk.instructions:
            si = inst.sync_info
            waits = list(si.on_wait) if si and si.on_wait else []
            if len(waits) > maxw:
                k = 0
                while len(waits) > maxw:
                    chunk, waits = waits[:maxw], waits[maxw:]
                    newlist.append(mybir.InstNoOp(
                        name=f"{inst.name}-waitsplit{k}", engine=inst.engine,
                        sync_info=mybir.SyncInfo(on_wait=chunk, on_update=[])))
                    k += 1
                inst.sync_info = mybir.SyncInfo(
                    on_wait=waits, on_update=list(si.on_update or []))
                changed = True
            newlist.append(inst)
        if changed:
            blk.instructions = newlist


def _layout():
    """Column layout of the packed [128, NF] f32 param tensor."""
    cols = {}
    off = 0

    def add(name, n):
        nonlocal off
        cols[name] = (off, off + n)
        off += n

    add("lnsel", 2)
    for l in range(N_LAYER):
        add(f"in_wT{l}", 4 * D_INNER)      # dir0 rows 0:64 cols 0:256; dir1 rows 64:128 cols 256:512
        for d in range(2):
            add(f"xbc{d}{l}", 2 * D_STATE)
            add(f"dtlin{d}{l}", D_INNER)
            add(f"out{d}{l}", D_MODEL)
            add(f"A{d}{l}", D_STATE)
            add(f"convw{d}{l}", D_CONV)
            add(f"convb{d}{l}", 1)
            add(f"dtb{d}{l}", 1)
            add(f"Dp{d}{l}", 1)
        add(f"nwb{l}", 1)
        add(f"nbb{l}", 1)
    add("nfwb", 1)
    add("nfbb", 1)
    add("poolw", 1)
    add("poolb", 1)
    add("llwT", D_MODEL)
    add("llb", 1)
    return cols, off


def build_nc(legalize=True):
    import concourse.bass as bass
    import concourse.mybir as mybir
    import concourse.tile as tile
    from contextlib import ExitStack

    dt32 = mybir.dt.float32
    dt16 = mybir.dt.bfloat16
    Alu = mybir.AluOpType
    Act = mybir.ActivationFunctionType

    cols, NF = _layout()

    nc = bass.Bass("TRN2", target_bir_lowering=False, debug=False,
                   num_devices=NCORES)

    xin = nc.dram_tensor("xin", [2 * D_MODEL, T], dt32, kind="ExternalInput").ap()
    pf_in = nc.dram_tensor("pf", [D_INNER, NF], dt32, kind="ExternalInput").ap()
    out_d = nc.dram_tensor("out", [D_MODEL, 1], dt32, kind="ExternalOutput").ap()

    # DRAM bounce scratch (reused; ordering guaranteed by same-queue FIFO)
    ln_dram = nc.dram_tensor("ln_scr", [2, 2 * T], dt16, kind="Internal").ap()
    bc_dram = nc.dram_tensor("bc_scr", [2 * D_STATE, T], dt16, kind="Internal").ap()
    a_dram = nc.dram_tensor("a_scr", [2, T], dt16, kind="Internal").ap()

    with tile.TileContext(nc) as tc, ExitStack() as ctx:
        const = ctx.enter_context(tc.tile_pool(name="const", bufs=1))
        pp = ctx.enter_context(tc.tile_pool(name="pp", bufs=2, space="PSUM"))

        PF = const.tile([D_INNER, NF], dt32, tag="pf")
        nc.sync.dma_start(out=PF, in_=pf_in)

        def P(name):
            s0, s1 = cols[name]
            return PF[:, s0:s1]

        eps_c = const.tile([D_INNER, 1], dt32, tag="eps")
        nc.vector.memset(eps_c, EPS)
        one_c = const.tile([D_INNER, 1], dt32, tag="one")
        nc.vector.memset(one_c, 1.0)
        carry = const.tile([D_INNER, D_STATE], dt32, tag="carry")

        # residual stream, both directions stacked
        res = const.tile([2 * D_MODEL, T], dt32, tag="res")
        nc.sync.dma_start(out=res, in_=xin)

        # per-dir conv inputs (pad cols stay zero across layers)
        xpad = [const.tile([D_INNER, D_CONV - 1 + T], dt32, tag=f"xpad{d}",
                           name=f"xpad{d}")
                for d in range(2)]
        for d in range(2):
            nc.vector.memset(xpad[d][:, 0:D_CONV - 1], 0.0)

        # ---- layernorm over features (partitions), both dirs at once ----
        def layer_norm(lp, src, nw_c, nb_c, hln):
            """src [128,T] f32 -> hln [128,T] f32 (both dirs); lp = scratch pool."""
            sq = lp.tile([2 * D_MODEL, T], dt32, tag="sq")
            nc.scalar.activation(sq, src, Act.Square)
            pstat = pp.tile([D_INNER, T], dt32, tag="pp")
            pm = pstat[0:2, :]
            psq = pp.tile([D_INNER, T], dt32, tag="pp", name="psq")[0:2, :]
            for j in range(T // MM):
                sj = slice(j * MM, (j + 1) * MM)
                nc.tensor.matmul(pm[:, sj], P("lnsel"), src[:, sj],
                                 start=True, stop=True)
            for j in range(T // MM):
                sj = slice(j * MM, (j + 1) * MM)
                nc.tensor.matmul(psq[:, sj], P("lnsel"), sq[:, sj],
                                 start=True, stop=True)
            r2 = lp.tile([2, 2 * T], dt16, tag="r2")      # [mean | rstd] rows
            msq = sq[0:2, :]                              # sq dead: reuse rows
            with nc.allow_low_precision("LN rows in bf16 feed DMA broadcast"):
                nc.scalar.activation(r2[:, 0:T], pm, Act.Copy)
                nc.scalar.activation(msq, pm, Act.Square)
                nc.vector.tensor_sub(msq, psq, msq)        # var
                nc.scalar.activation(msq, msq, Act.Sqrt, bias=eps_c[0:2, :])
                nc.vector.reciprocal(r2[:, T:2 * T], msq)
            nc.sync.dma_start(out=ln_dram, in_=r2)
            # broadcast rows: dir0 partitions 0:64 <- row0, dir1 <- row1
            mb = lp.tile([2 * D_MODEL, T], dt16, tag="mb")
            rb = lp.tile([2 * D_MODEL, T], dt16, tag="rb")
            src_m = ln_dram[:, 0:T].unsqueeze(1).to_broadcast([2, D_MODEL, T])
            src_r = ln_dram[:, T:2 * T].unsqueeze(1).to_broadcast([2, D_MODEL, T])
            nc.sync.dma_start(out=mb, in_=src_m)
            nc.sync.dma_start(out=rb, in_=src_r)
            nc.vector.tensor_sub(hln, src, mb)
            nc.vector.scalar_tensor_tensor(hln, hln, nw_c, rb,
                                           op0=Alu.mult, op1=Alu.mult)
            nc.scalar.activation(hln, hln, Act.Identity, bias=nb_c)

        # ---- one full layer (both dirs) ---------------------------------
        def layer(l, layp):
            zsilu = [layp.tile([D_INNER, T], dt32, tag=f"zsilu{d}",
                               name=f"zsilu{d}") for d in range(2)]
            iwT = P(f"in_wT{l}")
            with tc.tile_pool(name="lnp", bufs=1) as lnp:
                hln = lnp.tile([2 * D_MODEL, T], dt32, tag="hln")
                layer_norm(lnp, res, P(f"nwb{l}"), P(f"nbb{l}"), hln)
                for d in range(2):
                    hd = slice(d * D_MODEL, (d + 1) * D_MODEL)
                    co = 2 * D_INNER * d
                    # in_proj: x -> xpad[:,3:], z -> zsilu
                    px = pp.tile([D_INNER, T], dt32, tag="pp")
                    for j in range(T // MM):
                        sj = slice(j * MM, (j + 1) * MM)
                        nc.tensor.matmul(px[:, sj], iwT[hd, co:co + D_INNER],
                                         hln[hd, sj], start=True, stop=True)
                    nc.scalar.activation(xpad[d][:, D_CONV - 1:], px, Act.Copy)
                    pz = pp.tile([D_INNER, T], dt32, tag="pp", name="pz")
                    for j in range(T // MM):
                        sj = slice(j * MM, (j + 1) * MM)
                        nc.tensor.matmul(pz[:, sj],
                                         iwT[hd, co + D_INNER:co + 2 * D_INNER],
                                         hln[hd, sj], start=True, stop=True)
                    nc.scalar.activation(zsilu[d], pz, Act.Sigmoid)
                    nc.vector.tensor_mul(zsilu[d], zsilu[d], pz)

            for d in range(2):
                with tc.tile_pool(name="scanp", bufs=1) as sp:
                    hd = slice(d * D_MODEL, (d + 1) * D_MODEL)
                    # big scan tiles; dbxhs doubles as scan output (in-place)
                    bcb = sp.tile([D_INNER, D_STATE * L], dt16, tag="bcb")
                    dA = sp.tile([D_INNER, D_STATE * L], dt16, tag="dA")
                    dbxhs = sp.tile([D_INNER, D_STATE * L], dt16, tag="dbxhs")
                    dA3 = dA.rearrange("p (s l) -> p s l", s=D_STATE)
                    dbx3 = dbxhs.rearrange("p (s l) -> p s l", s=D_STATE)
                    bcb3 = bcb.rearrange("p (s l) -> p s l", s=D_STATE)
                    nc.vector.memset(dA3[:, :, 0], 0.0)    # gap cols: decay 0

                    # causal depthwise conv + silu (xsig borrows dbxhs space)
                    cw = P(f"convw{d}{l}")
                    xact = sp.tile([D_INNER, T], dt32, tag="xact")
                    nc.gpsimd.tensor_scalar(xact, xpad[d][:, 0:T], cw[:, 0:1],
                                          P(f"convb{d}{l}"), op0=Alu.mult,
                                          op1=Alu.add)
                    for jj in range(1, D_CONV):
                        nc.vector.scalar_tensor_tensor(
                            xact, xpad[d][:, jj:jj + T], cw[:, jj:jj + 1],
                            xact, op0=Alu.mult, op1=Alu.add)
                    xsig = dbxhs[:, :].bitcast(dt32)[:, 0:T]
                    nc.scalar.activation(xsig, xact, Act.Sigmoid)
                    nc.gpsimd.tensor_mul(xact, xact, xsig)

                    # xproj B/C rows -> bf16 -> DRAM (for DMA broadcast)
                    pbc = pp.tile([D_INNER, T], dt32, tag="pp",
                                  name="pbc")[0:2 * D_STATE, :]
                    for j in range(T // MM):
                        sj = slice(j * MM, (j + 1) * MM)
                        nc.tensor.matmul(pbc[:, sj], P(f"xbc{d}{l}"),
                                         xact[:, sj], start=True, stop=True)
                    bc16 = sp.tile([2 * D_STATE, T], dt16, tag="bc16")
                    with nc.allow_low_precision("B/C rows bf16 for broadcast"):
                        nc.scalar.activation(bc16, pbc, Act.Copy)
                    nc.sync.dma_start(out=bc_dram, in_=bc16)

                    # dt = softplus(dtlin @ xact + dt_b)
                    pdt = pp.tile([D_INNER, T], dt32, tag="pp", name="pdt")
                    for j in range(T // MM):
                        sj = slice(j * MM, (j + 1) * MM)
                        nc.tensor.matmul(pdt[:, sj], P(f"dtlin{d}{l}"),
                                         xact[:, sj], start=True, stop=True)
                    dts = sp.tile([D_INNER, T], dt32, tag="dts")
                    nc.scalar.activation(dts, pdt, Act.Exp, bias=P(f"dtb{d}{l}"))
                    nc.scalar.activation(dts, dts, Act.Ln, bias=one_c)

                    u = sp.tile([D_INNER, T], dt32, tag="u")
                    nc.vector.tensor_mul(u, dts, xact)

                    # ---- selective scan: 16 states per scan, gap-col resets
                    A_c = P(f"A{d}{l}")
                    yt = sp.tile([D_INNER, T], dt32, tag="yt")
                    nc.vector.memset(carry, 0.0)
                    bcbf = bcb[:, :].bitcast(dt32)       # tree scratch aliases
                    dbf = dbxhs[:, :].bitcast(dt32)
                    for c in range(NCH):
                        cs = slice(c * TCH, (c + 1) * TCH)
                        src_b = bc_dram[0:D_STATE, cs].partition_broadcast(D_INNER)
                        nc.sync.dma_start(out=bcb3[:, :, 1:], in_=src_b)
                        with nc.allow_low_precision("scan operands bf16"):
                            nc.vector.tensor_tensor(
                                dA3[:, :, 1:],
                                dts[:, cs].unsqueeze(1).to_broadcast(
                                    [D_INNER, D_STATE, TCH]),
                                A_c.unsqueeze(2).to_broadcast(
                                    [D_INNER, D_STATE, TCH]),
                                op=Alu.mult)
                            nc.scalar.activation(dA3[:, :, 1:], dA3[:, :, 1:],
                                                 Act.Exp)
                            nc.vector.tensor_tensor(
                                dbx3[:, :, 1:],
                                u[:, cs].unsqueeze(1).to_broadcast(
                                    [D_INNER, D_STATE, TCH]),
                                bcb3[:, :, 1:], op=Alu.mult)
                            nc.vector.tensor_copy(dbx3[:, :, 0], carry)
                            nc.vector.tensor_tensor_scan(dbxhs, dA, dbxhs, 0.0,
                                                         op0=Alu.mult,
                                                         op1=Alu.add)
                            if c < NCH - 1:
                                nc.vector.tensor_copy(carry, dbx3[:, :, L - 1])
                            src_c = bc_dram[D_STATE:2 * D_STATE, cs]\
                                .partition_broadcast(D_INNER)
                            nc.sync.dma_start(out=bcb3[:, :, 1:], in_=src_c)
                            ys = dA[:, 0:D_STATE * TCH]  # dA dead: reuse as ys
                            ys3 = ys.rearrange("p (s t) -> p s t", s=D_STATE)
                            nc.vector.tensor_tensor(ys3, dbx3[:, :, 1:],
                                                    bcb3[:, :, 1:], op=Alu.mult)
                        h8 = D_STATE * TCH // 2          # 8 segments worth
                        t1 = bcbf[:, 0:h8]
                        nc.gpsimd.tensor_add(t1, ys[:, 0:h8], ys[:, h8:2 * h8])
                        t2 = dbf[:, 0:h8 // 2]
                        nc.gpsimd.tensor_add(t2, t1[:, 0:h8 // 2], t1[:, h8 // 2:])
                        q = h8 // 4
                        t3 = dbf[:, h8 // 2:h8 // 2 + q]
                        nc.gpsimd.tensor_add(t3, t2[:, 0:q], t2[:, q:2 * q])
                        nc.gpsimd.tensor_add(yt[:, cs], t3[:, 0:q // 2],
                                           t3[:, q // 2:q])

                    # y = (xact*D + yt) * zsilu ; out_proj; residual update
                    y = u                                 # u dead: reuse
                    nc.vector.scalar_tensor_tensor(y, xact, P(f"Dp{d}{l}"), yt,
                                                 op0=Alu.mult, op1=Alu.add)
                    nc.gpsimd.tensor_mul(y, y, zsilu[d])
                    po = pp.tile([D_INNER, T], dt32, tag="pp",
                                 name="po")[0:D_MODEL, :]
                    for j in range(T // MM):
                        sj = slice(j * MM, (j + 1) * MM)
                        nc.tensor.matmul(po[:, sj], P(f"out{d}{l}"), y[:, sj],
                                         start=True, stop=True)
                    nc.vector.tensor_add(res[hd, :], po, res[hd, :])

        import os
        n_layers = int(os.environ.get("BK_LAYERS", N_LAYER))
        do_head = os.environ.get("BK_HEAD", "1") == "1"
        for l in range(n_layers):
            with tc.tile_pool(name="layp", bufs=1) as layp:
                layer(l, layp)

        # ---- head: final LN, softmax pool over T, linear ----------------
        if do_head:
            with tc.tile_pool(name="headp", bufs=1) as hp:
                hlnf = hp.tile([2 * D_MODEL, T], dt32, tag="hln")
                layer_norm(hp, res, P("nfwb"), P("nfbb"), hlnf)
                pooled = hp.tile([2 * D_MODEL, 1], dt32, tag="pooled")
                ttr_scr = hp.tile([2 * D_MODEL, T], dt32, tag="ttr_scr")
                a2 = hp.tile([2 * D_MODEL, T], dt16, tag="a2")
                for d in range(2):
                    hd = slice(d * D_MODEL, (d + 1) * D_MODEL)
                    pl = pp.tile([D_INNER, T], dt32, tag="pp",
                                 name="pl")[0:1, :]
                    for j in range(T // MM):
                        sj = slice(j * MM, (j + 1) * MM)
                        nc.tensor.matmul(pl[:, sj], P("poolw")[hd, :],
                                         hlnf[hd, sj], start=True, stop=True)
                    logits = hp.tile([1, T], dt32, tag="logits")
                    nc.scalar.activation(logits, pl, Act.Identity,
                                         bias=P("poolb")[64 * d:64 * d + 1, :])
                    smalls = hp.tile([1, 4], dt32, tag="smalls")
                    nc.vector.reduce_max(smalls[:, 0:1], logits,
                                         axis=mybir.AxisListType.X)
                    nc.vector.tensor_scalar_mul(smalls[:, 1:2],
                                                smalls[:, 0:1], -1.0)
                    nc.scalar.activation(logits, logits, Act.Exp,
                                         bias=smalls[:, 1:2])
                    nc.vector.reduce_sum(smalls[:, 2:3], logits,
                                         axis=mybir.AxisListType.X)
                    nc.vector.reciprocal(smalls[:, 3:4], smalls[:, 2:3])
                    with nc.allow_low_precision("softmax weights bf16"):
                        nc.vector.tensor_scalar(a2[64 * d:64 * d + 1, :],
                                                logits, smalls[:, 3:4],
                                                None, op0=Alu.mult)
                # both dirs' softmax rows -> DRAM -> broadcast -> weighted sum
                nc.sync.dma_start(
                    out=a_dram,
                    in_=a2.rearrange("(a p) t -> a p t", a=2)[:, 0, :])
                ab = hp.tile([2 * D_MODEL, T], dt16, tag="ab")
                nc.sync.dma_start(
                    out=ab,
                    in_=a_dram.unsqueeze(1).to_broadcast([2, D_MODEL, T]))
                nc.vector.tensor_mul(ttr_scr, hlnf, ab)
                nc.vector.reduce_sum(pooled, ttr_scr,
                                     axis=mybir.AxisListType.X)
                pout = pp.tile([D_INNER, T], dt32, tag="pp",
                               name="pout")[0:D_MODEL, 0:1]
                nc.tensor.matmul(pout, P("llwT"), pooled, start=True, stop=True)
                out_sb = hp.tile([D_MODEL, 1], dt32, tag="outsb")
                nc.scalar.activation(out_sb, pout, Act.Identity,
                                     bias=P("llb")[0:D_MODEL, :])
                nc.sync.dma_start(out=out_d, in_=out_sb)
        else:
            with tc.tile_pool(name="headp", bufs=1) as hp:
                out_sb = hp.tile([D_MODEL, 1], dt32, tag="outsb")
                nc.vector.tensor_copy(out_sb, res[0:D_MODEL, 0:1])
                nc.sync.dma_start(out=out_d, in_=out_sb)

    if legalize:
        _legalize_sync_waits(nc, mybir)
    return nc


def prep_inputs(inputs):
    """Host-side prep: pack params into one [128, NF] f32 tensor."""
    f = np.float32
    c = np.ascontiguousarray
    cols, NF = _layout()
    pf = np.zeros((D_INNER, NF), f)

    def put(name, block, rows=slice(0, D_INNER)):
        s0, s1 = cols[name]
        pf[rows, s0:s1] = block

    lnsel = np.zeros((D_INNER, 2), f)
    lnsel[0:D_MODEL, 0] = 1.0 / D_MODEL
    lnsel[D_MODEL:, 1] = 1.0 / D_MODEL
    put("lnsel", lnsel)

    in_w = np.asarray(inputs["in_w"], f)          # [2,4,256,64]
    xproj_w = np.asarray(inputs["xproj_w"], f)    # [2,4,36,128]
    dt_w = np.asarray(inputs["dt_w"], f)          # [2,4,128,4]
    out_w = np.asarray(inputs["out_w"], f)        # [2,4,64,128]
    A = -np.exp(np.asarray(inputs["A_log"], f))   # [2,4,128,16]
    conv_w = np.asarray(inputs["conv_w"], f)      # [2,4,128,4]
    nw = np.asarray(inputs["nw"], f)              # [2,4,64]
    nb = np.asarray(inputs["nb"], f)

    for l in range(N_LAYER):
        blk = np.zeros((D_INNER, 4 * D_INNER), f)
        blk[0:D_MODEL, 0:2 * D_INNER] = in_w[0, l].T
        blk[D_MODEL:, 2 * D_INNER:] = in_w[1, l].T
        put(f"in_wT{l}", blk)
        for d in range(2):
            put(f"xbc{d}{l}", xproj_w[d, l, DT_RANK:].T)          # [128,32]
            dtlin = dt_w[d, l] @ xproj_w[d, l, 0:DT_RANK]          # [128,128]
            put(f"dtlin{d}{l}", dtlin.T)
            put(f"out{d}{l}", out_w[d, l].T)
            put(f"A{d}{l}", A[d, l])
            put(f"convw{d}{l}", conv_w[d, l])
            put(f"convb{d}{l}", np.asarray(inputs["conv_b"], f)[d, l][:, None])
            put(f"dtb{d}{l}", np.asarray(inputs["dt_b"], f)[d, l][:, None])
            put(f"Dp{d}{l}", np.asarray(inputs["D"], f)[d, l][:, None])
        put(f"nwb{l}", np.concatenate([nw[0, l], nw[1, l]])[:, None])
        put(f"nbb{l}", np.concatenate([nb[0, l], nb[1, l]])[:, None])
    nf_w = np.asarray(inputs["nf_w"], f)
    nf_b = np.asarray(inputs["nf_b"], f)
    put("nfwb", np.concatenate([nf_w, nf_w])[:, None])
    put("nfbb", np.concatenate([nf_b, nf_b])[:, None])
    put("poolw", np.concatenate([np.asarray(inputs["fp_w"], f)[0],
                                 np.asarray(inputs["bp_w"], f)[0]])[:, None])
    poolb = np.zeros((D_INNER, 1), f)
    poolb[0, 0] = np.asarray(inputs["fp_b"], f)[0]
    poolb[64, 0] = np.asarray(inputs["bp_b"], f)[0]
    put("poolb", poolb)
    put("llwT", np.asarray(inputs["ll_w"], f).T)                   # [128,64]
    llb = np.zeros((D_INNER, 1), f)
    llb[0:D_MODEL, 0] = np.asarray(inputs["ll_b"], f)
    put("llb", llb)

    x = np.asarray(inputs["x"], f).reshape(B, D_MODEL, T)
    in_maps = []
    for b in range(B):
        m = {"pf": pf,
             "xin": c(np.concatenate([x[b], x[b, :, ::-1]], axis=0))}
        in_maps.append(m)
    return in_maps


def kernel(**inputs):
    from concourse.bass_utils import run_bass_kernel_spmd
    in_maps = prep_inputs(inputs)
    nc = build_nc()
    res = run_bass_kernel_spmd(nc, in_maps, core_ids=list(range(NCORES)))
    out = np.stack([res.results[b]["out"][:, 0] for b in range(B)])
    return out.astype(np.float32)
